# revision 2
# baseline (speedup 1.0000x reference)
"""AgentAttention Trainium2 kernel v3: 8-core data-parallel over batch.

Restructure vs baseline:
  - stage 2 scores computed transposed (keys on psum partitions) via
    k-stationary matmuls with a block-diagonal agent moving tile [128,113]
    (two heads packed at cols 0:49 / 64:113). No PE transposes, no big
    psum->sbuf copies; softmax sums land per-partition via v_ext ones col.
  - stage 3 pair-packed: bd-agT stationary -> 2 score matmuls/pair,
    pv as one [128,392] matmul + ones-matmul sums [2,392]; one fused
    normalize multiply per chunk.
  - stage 1: reciprocal reads psum directly (no [1,392] copies); the two
    heads' pv accumulation chains interleave across psum banks.
  - exps grouped into fewer, larger ACT ops.
PSUM tags: A=[128,1024]x2 (4 banks), P=[128,512]x2, C=[128,512]x2.
"""

import sys

sys.path.insert(0, "/opt/trn_rl_repo")

import numpy as np
import ml_dtypes

import concourse.bass as bass
import concourse.mybir as mybir
import concourse.tile as tile
from concourse import bacc, bass_utils

BF = mybir.dt.bfloat16
F32 = mybir.dt.float32
AF = mybir.ActivationFunctionType

N_CORES = 8
B, N, C = 32, 1176, 768
NB = B // N_CORES
H, HD = 12, 64
N_MT, N_S = 392, 784
A = 49
SCALE1 = 0.125
SCALE23 = 0.125 / 16.0

POS_T = [(pt * 128, min(128, N - pt * 128)) for pt in range(10)]
SIM_DEBUG = False
NCHUNK = [(0, 392), (392, 392), (784, 392)]


def build_program():
    nc = bacc.Bacc("TRN2", debug=False, num_devices=N_CORES)

    xT_d = nc.dram_tensor("xT", [NB, C, N], BF, kind="ExternalInput").ap()
    wqkT_d = nc.dram_tensor("wqkT", [C, 3 * C], BF, kind="ExternalInput").ap()
    wpjT_d = nc.dram_tensor("wpjT", [C, C], BF, kind="ExternalInput").ap()
    bqk_d = nc.dram_tensor("bqk", [1, 3 * C], BF, kind="ExternalInput").ap()
    bqkp_d = nc.dram_tensor("bqkp", [128, 12], F32, kind="ExternalInput").ap()
    bpj_d = nc.dram_tensor("bpj", [1, C], F32, kind="ExternalInput").ap()
    out_d = nc.dram_tensor("out", [NB, N, C], F32, kind="ExternalOutput").ap()

    with tile.TileContext(nc) as tc, nc.allow_low_precision(reason="attn bf16"):
        with (
            tc.tile_pool(name="const", bufs=1) as cpool,
            tc.tile_pool(name="work", bufs=2) as wpool,
            tc.tile_pool(name="hold", bufs=1) as hpool,
            tc.tile_pool(name="psum", bufs=2, space="PSUM") as ppool,
        ):
            # ---- one-time constants/weights ----
            wq = [
                cpool.tile([128, 3 * C], BF, tag=f"wq{i}", name=f"wq{i}")
                for i in range(6)
            ]
            wp = [
                cpool.tile([128, C], BF, tag=f"wp{i}", name=f"wp{i}") for i in range(6)
            ]
            for i in range(6):
                nc.sync.dma_start(wq[i][:], wqkT_d[128 * i : 128 * (i + 1), :])
                nc.sync.dma_start(wp[i][:], wpjT_d[128 * i : 128 * (i + 1), :])
            sb_bqk = cpool.tile([1, 3 * C], BF, tag="bqk")
            nc.sync.dma_start(sb_bqk[:], bqk_d[:])
            bqkp = cpool.tile([128, 12], F32, tag="bqkp")
            nc.sync.dma_start(bqkp[:], bqkp_d[:])
            vb_bc = cpool.tile([128, C], BF, tag="vb_bc")
            nc.gpsimd.partition_broadcast(vb_bc[:], sb_bqk[0:1, 2 * C : 3 * C])
            bpjf = cpool.tile([1, C], F32, tag="bpjf", name="bpjf")
            nc.sync.dma_start(bpjf[:], bpj_d[:])
            pb_bc = cpool.tile([128, C], F32, tag="pb_bc")
            nc.gpsimd.partition_broadcast(pb_bc[:], bpjf[0:1, :])
            # ones_bd for stage-3 sums: col0 = ones rows 0:49, col1 = ones 64:113
            ones_bd = cpool.tile([128, 2], BF, tag="ones_bd")
            nc.vector.memset(ones_bd[:], 0.0)
            nc.vector.memset(ones_bd[0:49, 0:1], 1.0)
            nc.vector.memset(ones_bd[64:113, 1:2], 1.0)

            for b in range(NB):
                # ---- load xT (bufs=2: batch b+1 loads overlap batch b) ----
                xT = [
                    wpool.tile([128, N], BF, tag=f"xT{i}", name=f"xT{i}", bufs=2)
                    for i in range(6)
                ]
                for i in range(6):
                    eng = nc.scalar if b == 0 else nc.sync
                    eng.dma_start(xT[i][:], xT_d[b, 128 * i : 128 * (i + 1), :])

                # ---- phase Q: qkT c-major (q,k rows), bias via vector TS add ----
                qkT = [None] * 12
                for m in [0, 6, 1, 7, 2, 8, 3, 9, 4, 10, 5, 11]:
                    ps = ppool.tile([128, 2 * 512], F32, tag="A", name="psA")
                    ps2 = ppool.tile([128, 512], F32, tag="P", name="psP")
                    for j, (n0, nsz) in enumerate(NCHUNK):
                        dst = ps[:, 512 * j : 512 * j + nsz] if j < 2 else ps2[:, 0:nsz]
                        for kt in range(6):
                            nc.tensor.matmul(
                                dst,
                                wq[kt][:, 128 * m : 128 * (m + 1)],
                                xT[kt][:, n0 : n0 + nsz],
                                start=(kt == 0),
                                stop=(kt == 5),
                            )
                    t = hpool.tile([128, N], BF, tag=f"qkT{m}", name=f"qkT{m}")
                    qkT[m] = t
                    nc.vector.tensor_scalar_add(
                        t[:, 0:784].rearrange("p (c x) -> p c x", c=2),
                        ps[:].rearrange("p (c x) -> p c x", c=2)[:, :, 0:392],
                        bqkp[:, m : m + 1],
                    )
                    nc.vector.tensor_scalar_add(
                        t[:, 784:1176], ps2[:, 0:392], bqkp[:, m : m + 1]
                    )

                # ---- phase V: pos-major v_ext with ones cols ----
                v_ext = []
                for pt, (p0, psz) in enumerate(POS_T):
                    ps = ppool.tile([128, 2 * 512], F32, tag="A", name="psA")
                    for c0, csz in [(0, 512), (512, 256)]:
                        for kt in range(6):
                            nc.tensor.matmul(
                                ps[0:psz, c0 : c0 + csz],
                                xT[kt][:, p0 : p0 + psz],
                                wq[kt][:, 2 * C + c0 : 2 * C + c0 + csz],
                                start=(kt == 0),
                                stop=(kt == 5),
                            )
                    vt = hpool.tile([128, H * 65], BF, tag=f"vx{pt}", name=f"vx{pt}")
                    v_ext.append(vt)
                    if b == 0 or SIM_DEBUG:
                        # bufs=1 slot memory persists across batches; evac only
                        # writes the 64 v columns, so ones survive
                        nc.vector.memset(
                            vt[:].rearrange("p (h e) -> p h e", e=65)[:, :, 64:65], 1.0
                        )
                    nc.vector.tensor_add(
                        vt[0:psz].rearrange("p (h e) -> p h e", e=65)[:, :, 0:64],
                        ps[0:psz, 0:768].rearrange("p (h d) -> p h d", d=64),
                        vb_bc[0:psz].rearrange("p (h d) -> p h d", d=64),
                    )
                    if pt == 3:
                        # replicated tail (pos 384:392) at partitions 32:40 for
                        # the merged stage-1 tail pv of the odd head
                        vt3r = hpool.tile([40, H * 65], BF, tag="vt3r", name="vt3r")
                        nc.vector.tensor_copy(vt3r[32:40, :], vt[0:8, :])

                # ---- agent pooling into block-diag bd-agT [128,113] ----
                # bd layout: head h block rows 0:64 x cols 0:49,
                #            head h' block rows 64:128 x cols 64:113
                agB = []
                for ct in range(6):
                    t1 = wpool.tile([128, 196], F32, tag="t1")
                    qs = qkT[ct][:, N_MT:N]  # [128, 784], idx = i*28 + aj*4 + dj
                    q4 = qs.rearrange("p (x dj) -> p x dj", dj=4)
                    nc.gpsimd.tensor_add(t1[:, 0:196], q4[:, :, 0:1], q4[:, :, 1:2])
                    nc.gpsimd.tensor_add(t1[:, 0:196], t1[:, 0:196], q4[:, :, 2:3])
                    nc.gpsimd.tensor_add(t1[:, 0:196], t1[:, 0:196], q4[:, :, 3:4])
                    bd = hpool.tile([128, 113], BF, tag=f"bd{ct}", name=f"bd{ct}")
                    agB.append(bd)
                    if b == 0 or SIM_DEBUG:
                        nc.vector.memset(bd[:], 0.0)
                    # t1 idx = 28*ai + 7*di + aj -> view (ai, aj, di)
                    t4 = t1[:, 0:196].rearrange("p (ai di aj) -> p ai aj di", ai=7, di=4)
                    t2 = wpool.tile([128, A], F32, tag="t2")
                    nc.gpsimd.tensor_add(t2[:, 0:A], t4[:, :, :, 0:1], t4[:, :, :, 1:2])
                    nc.gpsimd.tensor_add(t2[:, 0:A], t2[:, 0:A], t4[:, :, :, 2:3])
                    nc.gpsimd.tensor_add(
                        bd[0:64, 0:A], t2[0:64, 0:A], t4[0:64, :, :, 3:4]
                    )
                    nc.gpsimd.tensor_add(
                        bd[64:128, 64:113], t2[64:128, 0:A], t4[64:128, :, :, 3:4]
                    )

                # attention output accumulates here (c-major, bf16)
                aoT = [
                    hpool.tile([128, N], BF, tag=f"ao{i}", name=f"ao{i}")
                    for i in range(6)
                ]

                for p2 in range(6):
                    qt = p2
                    kq = qkT[qt]  # q pair tile
                    kk = qkT[6 + qt]  # k pair tile

                    # ---- stage 1 scores: psum A(kt0,kt1) P(kt2) C(kt3 tails) ----
                    scA = [None, None]
                    scP = [None, None]
                    for hp in range(2):
                        qo = 64 * hp
                        sa = ppool.tile([128, 2 * 512], F32, tag="A", name="psA")
                        sp = ppool.tile([128, 512], F32, tag="P", name="psP")
                        scA[hp], scP[hp] = sa, sp
                        for j, (k0, dst) in enumerate(
                            [(0, sa[0:128, 0:392]), (128, sa[0:128, 512:904]),
                             (256, sp[0:128, 0:392])]
                        ):
                            nc.tensor.matmul(
                                dst,
                                kk[qo : qo + 64, k0 : k0 + 128],
                                kq[qo : qo + 64, 0:N_MT],
                                start=True,
                                stop=True,
                            )
                    scC = ppool.tile([128, 512], F32, tag="C", name="psC")
                    for hp in range(2):
                        qo = 64 * hp
                        nc.tensor.matmul(
                            scC[32 * hp : 32 * hp + 8, 0:392],
                            kk[qo : qo + 64, 384:392],
                            kq[qo : qo + 64, 0:N_MT],
                            start=True,
                            stop=True,
                        )
                    # ---- stage 1 exps (grouped) ----
                    e1A = [
                        wpool.tile([128, 784], BF, tag="e1A", name="e1A", bufs=4)
                        for _ in range(2)
                    ]
                    e1P = [
                        wpool.tile([128, 392], BF, tag="e1P", name="e1P", bufs=4)
                        for _ in range(2)
                    ]
                    e1T = wpool.tile([40, 392], BF, tag="e1T", name="e1T", bufs=2)
                    for hp in range(2):
                        nc.scalar.activation(
                            e1A[hp][:].rearrange("p (c x) -> p c x", c=2),
                            scA[hp][:].rearrange("p (c x) -> p c x", c=2)[:, :, 0:392],
                            AF.Exp,
                            scale=SCALE1,
                        )
                        nc.scalar.activation(
                            e1P[hp][:, 0:392], scP[hp][:, 0:392], AF.Exp, scale=SCALE1
                        )
                    nc.scalar.activation(
                        e1T[0:8, 0:392], scC[0:8, 0:392], AF.Exp, scale=SCALE1
                    )
                    nc.scalar.activation(
                        e1T[32:40, 0:392], scC[32:40, 0:392], AF.Exp, scale=SCALE1
                    )
                    # ---- stage 1 pv: interleave heads across banks ----
                    pv1 = [
                        ppool.tile([128, 512], F32, tag="P", name="psP"),
                        ppool.tile([128, 512], F32, tag="C", name="psC"),
                    ]
                    for j in range(4):
                        for hp in range(2):
                            h = 2 * p2 + hp
                            if j < 3:
                                lhs = v_ext[j][0:128, 65 * h : 65 * h + 65]
                                rhs = [
                                    e1A[hp][:, 0:392],
                                    e1A[hp][:, 392:784],
                                    e1P[hp][:, 0:392],
                                ][j]
                            elif hp == 0:
                                lhs = v_ext[3][0:8, 65 * h : 65 * h + 65]
                                rhs = e1T[0:8, 0:392]
                            else:
                                # odd head's tail exp lives at partitions 32:40
                                lhs = vt3r[32:40, 65 * h : 65 * h + 65]
                                rhs = e1T[32:40, 0:392]
                            nc.tensor.matmul(
                                pv1[hp][0:65, 0:392],
                                lhs,
                                rhs,
                                start=(j == 0),
                                stop=(j == 3),
                                skip_group_check=True,
                            )
                    # ---- stage 1 normalize (partition bases must be 32-aligned;
                    # custom-DVE recip must read SBUF, so copy sums out first) ----
                    se1 = [
                        wpool.tile([1, 392], F32, tag=f"se1{hp}", name="se1", bufs=1)
                        for hp in range(2)
                    ]
                    rc1 = [
                        wpool.tile([1, 392], F32, tag=f"rc1{hp}", name="rc1", bufs=1)
                        for hp in range(2)
                    ]
                    # partition_broadcast silently corrupts when writing to
                    # partitions 64:128, so each head gets its own base-0 tile
                    bc1 = [
                        wpool.tile([64, 392], F32, tag=f"bc1{hp}", name="bc1", bufs=2)
                        for hp in range(2)
                    ]
                    for hp in range(2):
                        nc.vector.tensor_copy(
                            se1[hp][0:1, 0:392], pv1[hp][64:65, 0:392]
                        )
                        nc.vector.reciprocal_approx_fast(
                            out=rc1[hp][0:1, 0:392], in_=se1[hp][0:1, 0:392]
                        )
                        nc.gpsimd.partition_broadcast(
                            bc1[hp][0:64, :], rc1[hp][0:1, 0:392]
                        )
                        nc.vector.tensor_mul(
                            aoT[qt][64 * hp : 64 * hp + 64, 0:N_MT],
                            pv1[hp][0:64, 0:392],
                            bc1[hp][0:64, :],
                        )

                    # ---- stage 2: transposed scores, kt loop ----
                    e2T = wpool.tile([128, 10 * 113], BF, tag="e2T", name="e2T")
                    # emit score mms + grouped exps + pv accumulation interleaved
                    pv2 = ppool.tile([128, 512], F32, tag="C", name="psC")

                    def emit_pv2(kt_lo, kt_hi):
                        for kt in range(kt_lo, kt_hi):
                            p0, psz = POS_T[kt]
                            for hp in range(2):
                                h = 2 * p2 + hp
                                nc.tensor.matmul(
                                    pv2[64 * hp : 64 * hp + 49, 0:65],
                                    e2T[
                                        0:psz,
                                        113 * kt + 64 * hp : 113 * kt + 64 * hp + 49,
                                    ],
                                    v_ext[kt][0:psz, 65 * h : 65 * h + 65],
                                    start=(kt == 0),
                                    stop=(kt == 9),
                                    skip_group_check=True,
                                )

                    for ktp in range(5):
                        sa = ppool.tile([128, 2 * 512], F32, tag="A", name="psA")
                        for half in range(2):
                            kt = 2 * ktp + half
                            p0, psz = POS_T[kt]
                            nc.tensor.matmul(
                                sa[0:psz, 512 * half : 512 * half + 113],
                                kk[:, p0 : p0 + psz],
                                agB[qt][:, 0:113],
                                start=True,
                                stop=True,
                            )
                        if ktp < 4:
                            nc.scalar.activation(
                                e2T[:, 113 * 2 * ktp : 113 * (2 * ktp + 2)].rearrange(
                                    "p (c x) -> p c x", c=2
                                ),
                                sa[:].rearrange("p (c x) -> p c x", c=2)[:, :, 0:113],
                                AF.Exp,
                                scale=SCALE23,
                            )
                        else:
                            # kt9 has only 24 valid rows; keep its exp separate
                            nc.scalar.activation(
                                e2T[:, 113 * 8 : 113 * 9],
                                sa[:, 0:113],
                                AF.Exp,
                                scale=SCALE23,
                            )
                            nc.scalar.activation(
                                e2T[0:24, 113 * 9 : 113 * 10],
                                sa[0:24, 512:625],
                                AF.Exp,
                                scale=SCALE23,
                            )
                        if ktp > 0:
                            emit_pv2(2 * ktp - 2, 2 * ktp)
                    emit_pv2(8, 10)
                    # ---- av normalize into bd layout ----
                    rc2 = wpool.tile([128, 1], F32, tag="rc2", name="rc2", bufs=2)
                    avB = wpool.tile([128, 128], BF, tag="avB", name="avB", bufs=2)
                    if (b == 0 and p2 < 2) or SIM_DEBUG:
                        nc.vector.memset(avB[:], 0.0)
                    nc.vector.reciprocal(rc2[0:49, 0:1], pv2[0:49, 64:65])
                    nc.vector.reciprocal(rc2[64:113, 0:1], pv2[64:113, 64:65])
                    nc.scalar.activation(
                        avB[0:49, 0:64], pv2[0:49, 0:64], AF.Copy, scale=rc2[0:49, 0:1]
                    )
                    nc.scalar.activation(
                        avB[64:113, 64:128],
                        pv2[64:113, 0:64],
                        AF.Copy,
                        scale=rc2[64:113, 0:1],
                    )

                    # ---- stage 3 ----
                    scB = ppool.tile([128, 2 * 512], F32, tag="A", name="psA")
                    for cc in range(2):
                        nc.tensor.matmul(
                            scB[0:113, 512 * cc : 512 * cc + 392],
                            agB[qt][:, 0:113],
                            kq[:, N_MT + 392 * cc : N_MT + 392 * (cc + 1)],
                            start=True,
                            stop=True,
                        )
                    e3 = wpool.tile([128, 784], BF, tag="e3", name="e3", bufs=2)
                    nc.scalar.activation(
                        e3[0:113, :].rearrange("p (c x) -> p c x", c=2),
                        scB[0:113].rearrange("p (c x) -> p c x", c=2)[:, :, 0:392],
                        AF.Exp,
                        scale=SCALE23,
                    )
                    # per chunk: h sums at psum row 0, h' at row 64; one recip
                    # spans rows 0:65 (garbage rows 1:64 unused)
                    pv3 = [None, None]
                    sums3 = [None, None]
                    for cc in range(2):
                        pv3[cc] = ppool.tile([128, 512], F32, tag="P", name="psP")
                        sums3[cc] = ppool.tile([128, 512], F32, tag="C", name="psC")
                        # own rows 0:65 so the spanning reciprocal below reads
                        # no stale psum; recip(1.0) in unused rows is benign
                        nc.vector.memset(sums3[cc][0:65, 0:392], 1.0)
                        nc.tensor.matmul(
                            pv3[cc][0:128, 0:392],
                            avB[0:113, 0:128],
                            e3[0:113, 392 * cc : 392 * (cc + 1)],
                            start=True,
                            stop=True,
                        )
                        nc.tensor.matmul(
                            sums3[cc][0:1, 0:392],
                            ones_bd[0:113, 0:1],
                            e3[0:113, 392 * cc : 392 * (cc + 1)],
                            start=True,
                            stop=True,
                            skip_group_check=True,
                        )
                        nc.tensor.matmul(
                            sums3[cc][64:65, 0:392],
                            ones_bd[0:113, 1:2],
                            e3[0:113, 392 * cc : 392 * (cc + 1)],
                            start=True,
                            stop=True,
                            skip_group_check=True,
                        )
                    se3 = [
                        wpool.tile([65, 392], F32, tag=f"se3{cc}", name="se3", bufs=1)
                        for cc in range(2)
                    ]
                    rc3 = [
                        wpool.tile([65, 392], F32, tag=f"rc3{cc}", name="rc3", bufs=1)
                        for cc in range(2)
                    ]
                    for cc in range(2):
                        nc.vector.tensor_copy(
                            se3[cc][0:65, 0:392], sums3[cc][0:65, 0:392]
                        )
                        nc.vector.reciprocal_approx_fast(
                            out=rc3[cc][0:65, 0:392], in_=se3[cc][0:65, 0:392]
                        )
                        bc3 = [
                            wpool.tile(
                                [64, 392], F32, tag=f"bc3{hp}", name="bc3", bufs=2
                            )
                            for hp in range(2)
                        ]
                        # partition_broadcast misreads inputs at partition 64:
                        # bounce h' recip row to a base-0 tile first
                        rcx = wpool.tile([1, 392], F32, tag="rcx", name="rcx", bufs=2)
                        nc.vector.tensor_copy(rcx[0:1, 0:392], rc3[cc][64:65, 0:392])
                        nc.gpsimd.partition_broadcast(
                            bc3[0][0:64, :], rc3[cc][0:1, 0:392]
                        )
                        nc.gpsimd.partition_broadcast(
                            bc3[1][0:64, :], rcx[0:1, 0:392]
                        )
                        for hp in range(2):
                            nc.vector.tensor_mul(
                                aoT[qt][
                                    64 * hp : 64 * hp + 64,
                                    N_MT + 392 * cc : N_MT + 392 * (cc + 1),
                                ],
                                pv3[cc][64 * hp : 64 * hp + 64, 0:392],
                                bc3[hp][0:64, :],
                            )

                # ---- proj: out[pos, c] = aoT.T @ wpjT + bias ----
                for pt, (p0, psz) in enumerate(POS_T):
                    ps = ppool.tile([128, 2 * 512], F32, tag="A", name="psA")
                    for c0, csz in [(0, 512), (512, 256)]:
                        for kt in range(6):
                            nc.tensor.matmul(
                                ps[0:psz, c0 : c0 + csz],
                                aoT[kt][:, p0 : p0 + psz],
                                wp[kt][:, c0 : c0 + csz],
                                start=(kt == 0),
                                stop=(kt == 5),
                            )
                    ob = wpool.tile([128, C], F32, tag="osb")
                    nc.vector.tensor_add(ob[0:psz, :], ps[0:psz, 0:C], pb_bc[0:psz, :])
                    nc.sync.dma_start(out_d[b, p0 : p0 + psz, :], ob[0:psz, :])

    nc.compile()
    return nc


_PROGRAM = None


def _get_program():
    global _PROGRAM
    if _PROGRAM is None:
        _PROGRAM = build_program()
    return _PROGRAM


def _prep_maps(x, qkv_w, qkv_b, proj_w, proj_b):
    x = np.asarray(x, dtype=np.float32)
    bf = ml_dtypes.bfloat16
    xT = np.ascontiguousarray(x.transpose(0, 2, 1)).astype(bf)  # [B, C, N]
    wqkT = np.ascontiguousarray(np.asarray(qkv_w, dtype=np.float32).T).astype(bf)
    wpjT = np.ascontiguousarray(np.asarray(proj_w, dtype=np.float32).T).astype(bf)
    bqk = np.asarray(qkv_b, dtype=np.float32).reshape(1, -1).astype(bf)
    bqkp = np.ascontiguousarray(
        np.asarray(qkv_b, dtype=np.float32)[: 2 * 768].reshape(12, 128).T
    ).astype(np.float32)
    bpj = np.asarray(proj_b, dtype=np.float32).reshape(1, -1)
    return [
        {
            "xT": np.ascontiguousarray(xT[c * NB : (c + 1) * NB]),
            "wqkT": wqkT,
            "wpjT": wpjT,
            "bqk": bqk,
            "bqkp": bqkp,
            "bpj": bpj,
        }
        for c in range(N_CORES)
    ]


def kernel(x, qkv_w, qkv_b, proj_w, proj_b, t_h=14, t_w=14, s_h=28, s_w=28, **kw):
    nc = _get_program()
    in_maps = _prep_maps(x, qkv_w, qkv_b, proj_w, proj_b)
    res = bass_utils.run_bass_kernel_spmd(nc, in_maps, core_ids=list(range(N_CORES)))
    out = np.concatenate([res.results[c]["out"] for c in range(N_CORES)], axis=0)
    return out.astype(np.float32)


if __name__ == "__main__":
    build_program()
    print("program built OK")
